# revision 20
# baseline (speedup 1.0000x reference)
"""Trainium2 Bass kernel for 3x3 VALID conv (nn_BreakupConv).

x [16,64,128,128] f32, weights [128,64,9] f32 -> out [16,128,126,126] f32.

Scheme (v2c, "image-split row tiling, contiguous moving operand"):
- Data-parallel over 8 cores: 2 images per core.
- Per core, one SBUF x tile [128, 16384] fp16: partitions 0-63 hold image
  A's 64 channels, partitions 64-127 image B's. Every conv tap (ky,kx) is
  a K=64 matmul reading this UNSHIFTED buffer: image A on PE row group
  (0,0), image B on (64,0). Consecutive A/B matmuls use disjoint PE row
  halves and accumulate into different PSUM banks, so the hardware runs
  them concurrently (row tiling): a tap pair streams in one span. 9 taps
  covers TWO output tiles = the 4.5-span/128-row minimum for the 576-long
  contraction.
- The moving operand is a CONTIGUOUS 1D span of (r-1)*128+126 elements
  starting at flat offset (y+ky)*128+kx: it covers the r x 126 output
  window plus 6 junk columns in the inter-row gaps, positionally aligned
  across taps so PSUM accumulation stays correct. Contiguity matters: a
  strided 3D rhs ([64, r, 126] stride 128) streams ~2.3x slower on the PE
  (measured 88.7us vs 38us per rep in isolation); contiguous fp16 streams
  ~2 elements/cycle. The junk columns are dropped by the PSUM->SBUF copy
  (strided DVE read).
- PSUM banks drained to SBUF as fp16, DMA'd out in ~2 MiB groups (the
  final group's DMA is split in half so the tail overlaps the last
  copies). 16 warmup matmuls on a memset tile bridge the PE HAM
  clock-gate window during the initial x DMA (single-shot latency; they
  have no DMA dependency and are never read).
- I/O per core: 4 MiB in + 8.1 MiB out (fp16) vs 8.4 + 16.3 MiB for v1.
- Measured (R=199 paired-reps): ~29 us/rep vs ~190-210 us for v1 and for
  the strided variant; at the fp16 PE streaming peak (~30 us model,
  ~157 TFLOP/s effective) with DMA fully overlapped. fp8 would breach
  the 2e-2 error gate (measured 3.8e-2 host-side), so fp16 is final.
"""

import os
import numpy as np

os.environ.setdefault("BASS_NEVER_TRACE", "1")

B, C_IN, H, W = 16, 64, 128, 128
C_OUT, HO, WO = 128, 126, 126
N_CORES = 8
IMGS_PER_CORE = B // N_CORES  # 2
HW = H * W                    # 16384
ROWS_PER_TILE = 4             # output rows per PSUM tile (4*126 = 504 <= 512)
TAPS = 9

MM_DTYPE = os.environ.get("CONV_MM_DTYPE", "float16")
OUT_DTYPE = os.environ.get("CONV_OUT_DT", "float16")
# "alt": per tile pair, taps interleaved A,B,A,B -> relies on the PE
#   background weight buffer to hide per-matmul LDWEIGHTS.
# "tapgroup": tap-outer over TAPGROUP tile pairs -> same-weight matmuls
#   run back-to-back so weight (re)loads amortize over the group.
ORDER = os.environ.get("CONV_ORDER", "alt")
# "strided": matmul rhs is the 3D [64, r, 126] stride-128 window view.
# "contig": rhs is one contiguous span of (r-1)*128+126 elements covering
#   the window plus 6 junk columns (positionally consistent across taps);
#   the PSUM->SBUF copy drops the junk via a strided view. +1.2% PE cols,
#   but a pure 1D moving access pattern.
RHS_MODE = os.environ.get("CONV_RHS", "contig")
TAPGROUP = int(os.environ.get("CONV_TAPGROUP", "3"))
N_CHUNKS = int(os.environ.get("CONV_CHUNKS", "8"))
G_TILES = int(os.environ.get("CONV_G", "16"))   # row tiles per out DMA
N_WARM = int(os.environ.get("CONV_WARMUP", "16"))  # PE warmup matmuls
COPY_MODE = os.environ.get("CONV_COPY", "dve")  # "dve" | "split"
PS_BUFS = int(os.environ.get("CONV_PS_BUFS", "4"))   # PSUM bufs per image
OUT_BUFS = int(os.environ.get("CONV_OUT_BUFS", "2"))  # out bufs per image
# Timing-only ablation (breaks correctness): "noout" drops the output
# path (copies become 1-column live-sinks, no output DMAs).
ABLATE = os.environ.get("CONV_ABLATE", "")
# "pair": K=64 image-split row tiling (default). "dup128": v0-style
# shifted-dup layout (upper 64 partitions = image, lower = image shifted
# +1 col) with K=128 matmuls, 6 slots/tile, contiguous rhs — 2x MACs per
# streamed column; wins iff PE streaming is byte-limited rather than
# 2-elem/cycle-with-row-pairing.
SCHEME = os.environ.get("CONV_SCHEME", "pair")
# slot -> (read offset (ky,kx), upper tap, lower tap); lower half holds
# x shifted +1 col, so its effective tap is (ky, kx+1).
SLOTS128 = [
    ((0, 0), 0, 1), ((1, 0), 3, 4), ((2, 0), 6, 7),
    ((0, 2), 2, None), ((1, 2), 5, None), ((2, 2), 8, None),
]

_CACHE = {}


def _build_dup128(reps=1):
    """v0-style shifted-dup layout with K=128 matmuls and contiguous rhs."""
    import concourse.bacc as bacc
    import concourse.mybir as mybir
    from concourse.tile import TileContext

    dt = getattr(mybir.dt, MM_DTYPE)
    odt = getattr(mybir.dt, OUT_DTYPE)
    f32 = mybir.dt.float32
    NS = len(SLOTS128)

    nc = bacc.Bacc(None, target_bir_lowering=False)
    x_d = nc.dram_tensor("x2", [IMGS_PER_CORE, 128, HW], dt,
                         kind="ExternalInput")
    w_d = nc.dram_tensor("wmm", [128, NS * 128], dt, kind="ExternalInput")
    out_d = nc.dram_tensor("out2", [IMGS_PER_CORE, C_OUT, HO * WO], odt,
                           kind="ExternalOutput")

    with TileContext(nc) as tc:
        with (
            tc.tile_pool(name="xp", bufs=2) as xp,
            tc.tile_pool(name="wp", bufs=1) as wp,
            tc.tile_pool(name="pp", bufs=8, space="PSUM") as pp,
            tc.tile_pool(name="op", bufs=2 * OUT_BUFS) as op,
        ):
            w_sb = wp.tile([128, NS * 128], dt)
            nc.sync.dma_start(out=w_sb[:], in_=w_d[:])
            if N_WARM:
                warm_src = wp.tile([128, 504], dt, name="warm_src",
                                   tag="warm_src")
                nc.any.memset(warm_src[:], 0.0)
                warm_ps = pp.tile([128, 512], f32, name="warm", tag="ps",
                                  bufs=8)
                for _wi in range(N_WARM):
                    nc.tensor.matmul(
                        warm_ps[:, 0:504], warm_src[0:64, 0:128],
                        warm_src[0:64, 0:504], start=True, stop=True)
            for _rep in range(reps):
                for img in range(IMGS_PER_CORE):
                    x_sb = xp.tile([128, HW], dt, name="x_sb",
                                   tag=f"x{img}", bufs=1)
                    csz = HW // N_CHUNKS
                    for ci in range(N_CHUNKS):
                        nc.sync.dma_start(
                            out=x_sb[:, ci * csz:(ci + 1) * csz],
                            in_=x_d[img, :, ci * csz:(ci + 1) * csz])
                    for y0 in range(0, HO, G_TILES * ROWS_PER_TILE):
                        rg = min(G_TILES * ROWS_PER_TILE, HO - y0)
                        ot = op.tile([128, rg * WO], odt, name="ot",
                                     tag="ot", bufs=2 * OUT_BUFS)
                        ys = list(range(y0, y0 + rg, ROWS_PER_TILE))
                        for y in ys:
                            r = min(ROWS_PER_TILE, HO - y)
                            ps = pp.tile([128, 512], f32, name="ps",
                                         tag="ps", bufs=8)
                            n = (r - 1) * W + WO
                            for s, ((ky, kx), _tu, _tl) in enumerate(SLOTS128):
                                st = (y + ky) * W + kx
                                nc.tensor.matmul(
                                    ps[:, 0:n],
                                    w_sb[:, s * 128:(s + 1) * 128],
                                    x_sb[:, st:st + n],
                                    start=(s == 0), stop=(s == NS - 1))
                            oc = (y - y0) * WO
                            psv = ps[:].rearrange(
                                "p (h w) -> p h w", w=W)[:, 0:r, 0:WO]
                            nc.vector.tensor_copy(ot[:, oc:oc + r * WO], psv)
                        last_group = y0 + rg >= HO
                        if last_group and len(ys) > 1:
                            mid = ys[len(ys) // 2] - y0
                            nc.sync.dma_start(
                                out=out_d[img, :, y0 * WO:(y0 + mid) * WO],
                                in_=ot[:, 0:mid * WO])
                            nc.sync.dma_start(
                                out=out_d[img, :,
                                          (y0 + mid) * WO:(y0 + rg) * WO],
                                in_=ot[:, mid * WO:rg * WO])
                        else:
                            nc.sync.dma_start(
                                out=out_d[img, :, y0 * WO:(y0 + rg) * WO],
                                in_=ot[:])
    nc.compile()
    return nc


def _build_program(reps=1):
    import concourse.bacc as bacc
    import concourse.mybir as mybir
    from concourse.tile import TileContext

    if SCHEME == "dup128":
        return _build_dup128(reps)

    dt = getattr(mybir.dt, MM_DTYPE)
    odt = getattr(mybir.dt, OUT_DTYPE)
    f32 = mybir.dt.float32

    nc = bacc.Bacc(None, target_bir_lowering=False)
    x_d = nc.dram_tensor("x2", [128, HW], dt, kind="ExternalInput")
    w_d = nc.dram_tensor("wmm", [128, TAPS * 128], dt, kind="ExternalInput")
    out_d = nc.dram_tensor("out2", [IMGS_PER_CORE, C_OUT, HO * WO], odt,
                           kind="ExternalOutput")

    with TileContext(nc) as tc:
        with (
            tc.tile_pool(name="xp", bufs=2) as xp,
            tc.tile_pool(name="wp", bufs=1) as wp,
            tc.tile_pool(name="pp", bufs=2 * PS_BUFS, space="PSUM") as pp,
            tc.tile_pool(name="op", bufs=2 * OUT_BUFS) as op,
        ):
            w_sb = wp.tile([128, TAPS * 128], dt)
            nc.sync.dma_start(out=w_sb[:], in_=w_d[:])
            if N_WARM:
                # Warm the PE HAM clock gate during the initial x DMA: matmuls
                # on a memset tile (no DMA dependency), results never read.
                warm_src = wp.tile([128, 504], dt, name="warm_src",
                                   tag="warm_src")
                nc.any.memset(warm_src[:], 0.0)
                warm_ps = pp.tile([128, 512], f32, name="warm", tag="ps0",
                                  bufs=PS_BUFS)
                for _wi in range(N_WARM):
                    nc.tensor.matmul(
                        warm_ps[:, 0:504], warm_src[0:64, 0:128],
                        warm_src[0:64, 0:504], start=True, stop=True)

            def copy0(ot_slice, ps_view):
                nc.vector.tensor_copy(ot_slice, ps_view)

            def copy1(ot_slice, ps_view):
                if COPY_MODE == "split":
                    nc.scalar.copy(ot_slice, ps_view)
                else:
                    nc.vector.tensor_copy(ot_slice, ps_view)

            for _rep in range(reps):
                x_sb = xp.tile([128, HW], dt, name="x_sb", tag="x")
                csz = HW // N_CHUNKS
                for ci in range(N_CHUNKS):
                    nc.sync.dma_start(
                        out=x_sb[:, ci * csz:(ci + 1) * csz],
                        in_=x_d[:, ci * csz:(ci + 1) * csz])
                xv = x_sb[:].rearrange("p (h w) -> p h w", h=H)

                def mm(ps, img, t, y, r):
                    ky, kx = divmod(t, 3)
                    p0 = img * 64
                    if RHS_MODE == "contig":
                        n = (r - 1) * W + WO
                        s = (y + ky) * W + kx
                        nc.tensor.matmul(
                            ps[:, 0:n],
                            w_sb[p0:p0 + 64, t * 128:(t + 1) * 128],
                            x_sb[p0:p0 + 64, s:s + n],
                            start=(t == 0), stop=(t == TAPS - 1),
                        )
                    else:
                        nc.tensor.matmul(
                            ps[:],
                            w_sb[p0:p0 + 64, t * 128:(t + 1) * 128],
                            xv[p0:p0 + 64, y + ky:y + ky + r, kx:kx + WO],
                            start=(t == 0), stop=(t == TAPS - 1),
                        )

                def ps_alloc(pool_tag, r, bufs):
                    if RHS_MODE == "contig":
                        t_ = pp.tile([128, 512], f32, name=pool_tag,
                                     tag=pool_tag, bufs=bufs)
                        return t_
                    return pp.tile([128, r * WO], f32, name=pool_tag,
                                   tag=pool_tag, bufs=bufs)

                def ps_read(ps, r):
                    if RHS_MODE == "contig":
                        return ps[:].rearrange(
                            "p (h w) -> p h w", w=W)[:, 0:r, 0:WO]
                    return ps[:]

                if ABLATE == "noout":
                    live = op.tile([128, 512], f32, name="live", tag="live")
                    k = 0
                    for y in range(0, HO, ROWS_PER_TILE):
                        r = min(ROWS_PER_TILE, HO - y)
                        ps0 = ps_alloc("ps0", r, PS_BUFS)
                        ps1 = ps_alloc("ps1", r, PS_BUFS)
                        for t in range(TAPS):
                            mm(ps0, 0, t, y, r)
                            mm(ps1, 1, t, y, r)
                        nc.vector.tensor_copy(live[:, k % 512:k % 512 + 1],
                                              ps0[:, 0:1])
                        k += 1
                        nc.vector.tensor_copy(live[:, k % 512:k % 512 + 1],
                                              ps1[:, 0:1])
                        k += 1
                    continue
                for y0 in range(0, HO, G_TILES * ROWS_PER_TILE):
                    rg = min(G_TILES * ROWS_PER_TILE, HO - y0)
                    ots = [op.tile([128, rg * WO], odt, name=f"ot{i}",
                                   tag=f"ot{i}", bufs=OUT_BUFS)
                           for i in range(IMGS_PER_CORE)]
                    ys = list(range(y0, y0 + rg, ROWS_PER_TILE))
                    if ORDER == "alt":
                        for y in ys:
                            r = min(ROWS_PER_TILE, HO - y)
                            ps0 = ps_alloc("ps0", r, PS_BUFS)
                            ps1 = ps_alloc("ps1", r, PS_BUFS)
                            for t in range(TAPS):
                                mm(ps0, 0, t, y, r)
                                mm(ps1, 1, t, y, r)
                            oc = (y - y0) * WO
                            copy0(ots[0][:, oc:oc + r * WO], ps_read(ps0, r))
                            copy1(ots[1][:, oc:oc + r * WO], ps_read(ps1, r))
                    else:  # tapgroup
                        for gi in range(0, len(ys), TAPGROUP):
                            sub = ys[gi:gi + TAPGROUP]
                            pairs = []
                            for y in sub:
                                r = min(ROWS_PER_TILE, HO - y)
                                ps0 = ps_alloc("ps0", r, PS_BUFS)
                                ps1 = ps_alloc("ps1", r, PS_BUFS)
                                pairs.append((y, r, ps0, ps1))
                            for t in range(TAPS):
                                for (y, r, ps0, ps1) in pairs:
                                    mm(ps0, 0, t, y, r)
                                    mm(ps1, 1, t, y, r)
                            for (y, r, ps0, ps1) in pairs:
                                oc = (y - y0) * WO
                                copy0(ots[0][:, oc:oc + r * WO],
                                      ps_read(ps0, r))
                                copy1(ots[1][:, oc:oc + r * WO],
                                      ps_read(ps1, r))
                    last_group = y0 + rg >= HO  # noqa: E501  (ablation skips via continue above)
                    for i in range(IMGS_PER_CORE):
                        if last_group and len(ys) > 1:
                            # split the final out DMA so its first half
                            # overlaps the last tiles' copies (shorter tail)
                            mid = ys[len(ys) // 2] - y0
                            nc.sync.dma_start(
                                out=out_d[i, :, y0 * WO:(y0 + mid) * WO],
                                in_=ots[i][:, 0:mid * WO])
                            nc.sync.dma_start(
                                out=out_d[i, :, (y0 + mid) * WO:(y0 + rg) * WO],
                                in_=ots[i][:, mid * WO:rg * WO])
                        else:
                            nc.sync.dma_start(
                                out=out_d[i, :, y0 * WO:(y0 + rg) * WO],
                                in_=ots[i][:])
            if ABLATE == "noout":
                # satisfy the ExternalOutput with one token write
                tok = op.tile([128, 512], odt, name="tok", tag="tok")
                nc.any.memset(tok[:], 0.0)
                nc.sync.dma_start(out=out_d[0, :, 0:512], in_=tok[:])
    nc.compile()
    return nc


def _build_nop_program():
    """Minimal program with the same I/O contract, for dispatch-floor timing."""
    import concourse.bacc as bacc
    import concourse.mybir as mybir
    from concourse.tile import TileContext

    f32 = mybir.dt.float32
    nc = bacc.Bacc(None, target_bir_lowering=False)
    x_d = nc.dram_tensor("xn", [128, 128], f32, kind="ExternalInput")
    o_d = nc.dram_tensor("on", [128, 128], f32, kind="ExternalOutput")
    with TileContext(nc) as tc:
        with tc.tile_pool(name="p", bufs=1) as p:
            t = p.tile([128, 128], f32)
            nc.sync.dma_start(out=t[:], in_=x_d[:])
            nc.sync.dma_start(out=o_d[:], in_=t[:])
    nc.compile()
    return nc


def _make_runner(nc):
    """Build a reusable jitted SPMD callable for `nc` over 8 cores.

    Returns (run, meta): run(list_of_global_np_inputs) -> list of global
    np outputs with shape (N_CORES*dim0, ...). Inputs are device_put once
    per call; no donation (outputs fully written by the kernel).
    """
    import jax
    import concourse.mybir as mybir
    from concourse import bass2jax
    from jax.experimental.shard_map import shard_map
    from jax.sharding import Mesh, NamedSharding, PartitionSpec

    bass2jax.install_neuronx_cc_hook()

    partition_name = (
        nc.partition_id_tensor.name if nc.partition_id_tensor is not None else None
    )
    in_names, out_names, out_avals, zero_outs = [], [], [], []
    for alloc in nc.m.functions[0].allocations:
        if not isinstance(alloc, mybir.MemoryLocationSet):
            continue
        name = alloc.memorylocations[0].name
        if alloc.kind == "ExternalInput":
            if name != partition_name:
                in_names.append(name)
        elif alloc.kind == "ExternalOutput":
            out_names.append(name)
            shape = tuple(alloc.tensor_shape)
            dtype = mybir.dt.np(alloc.dtype)
            out_avals.append(jax.core.ShapedArray(shape, dtype))
            zero_outs.append(np.zeros(shape, dtype))
    n_params = len(in_names)
    all_in_names = list(in_names) + list(out_names)
    if partition_name is not None:
        all_in_names.append(partition_name)

    def _body(*args):
        operands = list(args)
        if partition_name is not None:
            operands.append(bass2jax.partition_id_tensor())
        outs = bass2jax._bass_exec_p.bind(
            *operands,
            out_avals=tuple(out_avals),
            in_names=tuple(all_in_names),
            out_names=tuple(out_names),
            lowering_input_output_aliases=(),
            sim_require_finite=True,
            sim_require_nnan=True,
            nc=nc,
        )
        return tuple(outs)

    devices = jax.devices()[:N_CORES]
    mesh = Mesh(np.asarray(devices), ("core",))
    spec = PartitionSpec("core")
    n_args = n_params + len(out_names)
    sharded = jax.jit(
        shard_map(
            _body,
            mesh=mesh,
            in_specs=(spec,) * n_args,
            out_specs=(spec,) * len(out_names),
            check_rep=False,
        ),
        keep_unused=True,
    )
    sharding = NamedSharding(mesh, spec)
    zeros_dev = [
        jax.device_put(np.zeros((N_CORES * z.shape[0], *z.shape[1:]), z.dtype),
                       sharding)
        for z in zero_outs
    ]

    def run(global_inputs, device_inputs=None):
        if device_inputs is None:
            device_inputs = [jax.device_put(g, sharding) for g in global_inputs]
        outs = sharded(*device_inputs, *zeros_dev)
        jax.block_until_ready(outs)
        return outs

    meta = {
        "sharding": sharding,
        "out_avals": out_avals,
        "out_names": out_names,
        "jax": jax,
        "sharded": sharded,
        "zeros_dev": zeros_dev,
    }
    return run, meta


def get_runner(reps=1):
    key = ("runner", reps)
    if key not in _CACHE:
        nc = _build_program(reps)
        _CACHE[key] = _make_runner(nc)
    return _CACHE[key]


def get_nop_runner():
    if "nop_runner" not in _CACHE:
        nc = _build_nop_program()
        _CACHE["nop_runner"] = _make_runner(nc)
    return _CACHE["nop_runner"]


def _np_mm_dtype():
    return {"float16": np.float16, "bfloat16": None}.get(MM_DTYPE, np.float32)


def prep_inputs(x, weights):
    """Host-side shard prep: returns global (concat over cores) inputs."""
    npdt = _np_mm_dtype()
    if npdt is None:
        import ml_dtypes
        npdt = ml_dtypes.bfloat16
    x = np.asarray(x, dtype=np.float32).reshape(B, C_IN, HW).astype(npdt)
    w = np.asarray(weights, dtype=np.float32).astype(npdt)  # [128, 64, 9]

    if SCHEME == "dup128":
        # per image: upper 64 partitions = channels, lower 64 = shifted +1
        x2 = np.zeros((B, 2 * C_IN, HW), npdt)
        x2[:, :C_IN, :] = x
        x2[:, C_IN:, :HW - 1] = x[:, :, 1:]
        NS = len(SLOTS128)
        wmm = np.zeros((128, NS * 128), npdt)
        for s, (_off, tu, tl) in enumerate(SLOTS128):
            wmm[0:64, s * 128:(s + 1) * 128] = w[:, :, tu].T
            if tl is not None:
                wmm[64:128, s * 128:(s + 1) * 128] = w[:, :, tl].T
        wmm_global = np.tile(wmm, (N_CORES, 1))
        # x_d is [2, 128, HW] per core -> global keeps trailing dims:
        # [16, 128, HW], dim0 sharded 8-ways into pairs of images
        return [x2, wmm_global]

    # core c holds images 2c (partitions 0-63) and 2c+1 (64-127)
    x2_global = np.ascontiguousarray(
        x.reshape(N_CORES, IMGS_PER_CORE * C_IN, HW))
    wmm = np.zeros((128, TAPS * 128), npdt)
    for t in range(TAPS):
        wT = w[:, :, t].T  # [64, 128]
        wmm[0:64, t * 128:(t + 1) * 128] = wT
        wmm[64:128, t * 128:(t + 1) * 128] = wT
    wmm_global = np.tile(wmm, (N_CORES, 1))  # [8*128, 1152]
    return [x2_global.reshape(N_CORES * 2 * C_IN, HW), wmm_global]


def kernel(x, weights):
    run, _meta = get_runner()
    outs = run(prep_inputs(x, weights))
    out_g = np.asarray(outs[0])  # [16, 128, HO*WO] in OUT_DTYPE
    return out_g.reshape(B, C_OUT, HO, WO).astype(np.float32)


# revision 23
# speedup vs baseline: 1.2538x; 1.2538x over previous
"""Trainium2 Bass kernel for 3x3 VALID conv (nn_BreakupConv).

x [16,64,128,128] f32, weights [128,64,9] f32 -> out [16,128,126,126] f32.

Scheme (v2c, "image-split row tiling, contiguous moving operand"):
- Data-parallel over 8 cores: 2 images per core.
- Per core, one SBUF x tile [128, 16384] fp16: partitions 0-63 hold image
  A's 64 channels, partitions 64-127 image B's. Every conv tap (ky,kx) is
  a K=64 matmul reading this UNSHIFTED buffer: image A on PE row group
  (0,0), image B on (64,0). Consecutive A/B matmuls use disjoint PE row
  halves and accumulate into different PSUM banks, so the hardware runs
  them concurrently (row tiling): a tap pair streams in one span. 9 taps
  covers TWO output tiles = the 4.5-span/128-row minimum for the 576-long
  contraction.
- The moving operand is a CONTIGUOUS 1D span of (r-1)*128+126 elements
  starting at flat offset (y+ky)*128+kx: it covers the r x 126 output
  window plus 6 junk columns in the inter-row gaps, positionally aligned
  across taps so PSUM accumulation stays correct. Contiguity matters: a
  strided 3D rhs ([64, r, 126] stride 128) streams ~2.3x slower on the PE
  (measured 88.7us vs 38us per rep in isolation); contiguous fp16 streams
  ~2 elements/cycle. The junk columns are dropped by the PSUM->SBUF copy
  (strided DVE read).
- PSUM banks drained to SBUF as fp16, DMA'd out in ~2 MiB groups (the
  final group's DMA is split in half so the tail overlaps the last
  copies). 16 warmup matmuls on a memset tile bridge the PE HAM
  clock-gate window during the initial x DMA (single-shot latency; they
  have no DMA dependency and are never read).
- I/O per core: 4 MiB in + 8.1 MiB out (fp16) vs 8.4 + 16.3 MiB for v1.
- Measured (R=199 paired-reps): ~29 us/rep vs ~190-210 us for v1 and for
  the strided variant; at the fp16 PE streaming peak (~30 us model,
  ~157 TFLOP/s effective) with DMA fully overlapped. fp8 would breach
  the 2e-2 error gate (measured 3.8e-2 host-side), so fp16 is final.
"""

import os
import numpy as np

os.environ.setdefault("BASS_NEVER_TRACE", "1")

B, C_IN, H, W = 16, 64, 128, 128
C_OUT, HO, WO = 128, 126, 126
N_CORES = 8
IMGS_PER_CORE = B // N_CORES  # 2
HW = H * W                    # 16384
ROWS_PER_TILE = 4             # output rows per PSUM tile (4*126 = 504 <= 512)
TAPS = 9

MM_DTYPE = os.environ.get("CONV_MM_DTYPE", "float16")
OUT_DTYPE = os.environ.get("CONV_OUT_DT", "float16")
# "alt": per tile pair, taps interleaved A,B,A,B -> relies on the PE
#   background weight buffer to hide per-matmul LDWEIGHTS.
# "tapgroup": tap-outer over TAPGROUP tile pairs -> same-weight matmuls
#   run back-to-back so weight (re)loads amortize over the group.
ORDER = os.environ.get("CONV_ORDER", "alt")
# "strided": matmul rhs is the 3D [64, r, 126] stride-128 window view.
# "contig": rhs is one contiguous span of (r-1)*128+126 elements covering
#   the window plus 6 junk columns (positionally consistent across taps);
#   the PSUM->SBUF copy drops the junk via a strided view. +1.2% PE cols,
#   but a pure 1D moving access pattern.
RHS_MODE = os.environ.get("CONV_RHS", "contig")
TAPGROUP = int(os.environ.get("CONV_TAPGROUP", "3"))
N_CHUNKS = int(os.environ.get("CONV_CHUNKS", "8"))
G_TILES = int(os.environ.get("CONV_G", "16"))   # row tiles per out DMA
N_WARM = int(os.environ.get("CONV_WARMUP", "16"))  # PE warmup matmuls
COPY_MODE = os.environ.get("CONV_COPY", "dve")  # "dve" | "split"
PS_BUFS = int(os.environ.get("CONV_PS_BUFS", "4"))   # PSUM bufs per image
OUT_BUFS = int(os.environ.get("CONV_OUT_BUFS", "2"))  # out bufs per image
# Timing-only ablation (breaks correctness): "noout" drops the output
# path (copies become 1-column live-sinks, no output DMAs).
ABLATE = os.environ.get("CONV_ABLATE", "")
# "pair": K=64 image-split row tiling (default). "dup128": v0-style
# shifted-dup layout (upper 64 partitions = image, lower = image shifted
# +1 col) with K=128 matmuls, 6 slots/tile, contiguous rhs. Measured
# (R=199): dup128 = 80.8 us/rep vs pair = 28.3 — a K=128 matmul streams
# ~1 elem/cycle (~210 ns/MM) while K=64 row-tiled pairs reach ~49 ns
# effective per MM (2-elem/cycle fp16 streaming x 2 concurrent row
# groups). Keep "pair".
SCHEME = os.environ.get("CONV_SCHEME", "pair")
# slot -> (read offset (ky,kx), upper tap, lower tap); lower half holds
# x shifted +1 col, so its effective tap is (ky, kx+1).
SLOTS128 = [
    ((0, 0), 0, 1), ((1, 0), 3, 4), ((2, 0), 6, 7),
    ((0, 2), 2, None), ((1, 2), 5, None), ((2, 2), 8, None),
]

_CACHE = {}


def _build_dup128(reps=1):
    """v0-style shifted-dup layout with K=128 matmuls and contiguous rhs."""
    import concourse.bacc as bacc
    import concourse.mybir as mybir
    from concourse.tile import TileContext

    dt = getattr(mybir.dt, MM_DTYPE)
    odt = getattr(mybir.dt, OUT_DTYPE)
    f32 = mybir.dt.float32
    NS = len(SLOTS128)

    nc = bacc.Bacc(None, target_bir_lowering=False)
    x_d = nc.dram_tensor("x2", [IMGS_PER_CORE, 128, HW], dt,
                         kind="ExternalInput")
    w_d = nc.dram_tensor("wmm", [128, NS * 128], dt, kind="ExternalInput")
    out_d = nc.dram_tensor("out2", [IMGS_PER_CORE, C_OUT, HO * WO], odt,
                           kind="ExternalOutput")

    with TileContext(nc) as tc:
        with (
            tc.tile_pool(name="xp", bufs=2) as xp,
            tc.tile_pool(name="wp", bufs=1) as wp,
            tc.tile_pool(name="pp", bufs=8, space="PSUM") as pp,
            tc.tile_pool(name="op", bufs=2 * OUT_BUFS) as op,
        ):
            w_sb = wp.tile([128, NS * 128], dt)
            nc.sync.dma_start(out=w_sb[:], in_=w_d[:])
            if N_WARM:
                warm_src = wp.tile([128, 504], dt, name="warm_src",
                                   tag="warm_src")
                nc.any.memset(warm_src[:], 0.0)
                warm_ps = pp.tile([128, 512], f32, name="warm", tag="ps",
                                  bufs=8)
                for _wi in range(N_WARM):
                    nc.tensor.matmul(
                        warm_ps[:, 0:504], warm_src[0:64, 0:128],
                        warm_src[0:64, 0:504], start=True, stop=True)
            for _rep in range(reps):
                for img in range(IMGS_PER_CORE):
                    x_sb = xp.tile([128, HW], dt, name="x_sb",
                                   tag=f"x{img}", bufs=1)
                    csz = HW // N_CHUNKS
                    for ci in range(N_CHUNKS):
                        nc.sync.dma_start(
                            out=x_sb[:, ci * csz:(ci + 1) * csz],
                            in_=x_d[img, :, ci * csz:(ci + 1) * csz])
                    for y0 in range(0, HO, G_TILES * ROWS_PER_TILE):
                        rg = min(G_TILES * ROWS_PER_TILE, HO - y0)
                        ot = op.tile([128, rg * WO], odt, name="ot",
                                     tag="ot", bufs=2 * OUT_BUFS)
                        ys = list(range(y0, y0 + rg, ROWS_PER_TILE))
                        for y in ys:
                            r = min(ROWS_PER_TILE, HO - y)
                            ps = pp.tile([128, 512], f32, name="ps",
                                         tag="ps", bufs=8)
                            n = (r - 1) * W + WO
                            for s, ((ky, kx), _tu, _tl) in enumerate(SLOTS128):
                                st = (y + ky) * W + kx
                                nc.tensor.matmul(
                                    ps[:, 0:n],
                                    w_sb[:, s * 128:(s + 1) * 128],
                                    x_sb[:, st:st + n],
                                    start=(s == 0), stop=(s == NS - 1))
                            oc = (y - y0) * WO
                            psv = ps[:].rearrange(
                                "p (h w) -> p h w", w=W)[:, 0:r, 0:WO]
                            nc.vector.tensor_copy(ot[:, oc:oc + r * WO], psv)
                        last_group = y0 + rg >= HO
                        if last_group and len(ys) > 1:
                            mid = ys[len(ys) // 2] - y0
                            nc.sync.dma_start(
                                out=out_d[img, :, y0 * WO:(y0 + mid) * WO],
                                in_=ot[:, 0:mid * WO])
                            nc.sync.dma_start(
                                out=out_d[img, :,
                                          (y0 + mid) * WO:(y0 + rg) * WO],
                                in_=ot[:, mid * WO:rg * WO])
                        else:
                            nc.sync.dma_start(
                                out=out_d[img, :, y0 * WO:(y0 + rg) * WO],
                                in_=ot[:])
    nc.compile()
    return nc


def _build_program(reps=1):
    import concourse.bacc as bacc
    import concourse.mybir as mybir
    from concourse.tile import TileContext

    if SCHEME == "dup128":
        return _build_dup128(reps)

    dt = getattr(mybir.dt, MM_DTYPE)
    odt = getattr(mybir.dt, OUT_DTYPE)
    f32 = mybir.dt.float32

    nc = bacc.Bacc(None, target_bir_lowering=False)
    x_d = nc.dram_tensor("x2", [128, HW], dt, kind="ExternalInput")
    w_d = nc.dram_tensor("wmm", [128, TAPS * 128], dt, kind="ExternalInput")
    out_d = nc.dram_tensor("out2", [IMGS_PER_CORE, C_OUT, HO * WO], odt,
                           kind="ExternalOutput")

    with TileContext(nc) as tc:
        with (
            tc.tile_pool(name="xp", bufs=2) as xp,
            tc.tile_pool(name="wp", bufs=1) as wp,
            tc.tile_pool(name="pp", bufs=2 * PS_BUFS, space="PSUM") as pp,
            tc.tile_pool(name="op", bufs=2 * OUT_BUFS) as op,
        ):
            w_sb = wp.tile([128, TAPS * 128], dt)
            # ACT HWDGE ring (qActDynamicHW): keeps the weights load and all
            # output DMAs off the SP ring's FIFO so input chunks are never
            # head-of-line blocked behind 2 MiB output transfers.
            nc.scalar.dma_start(out=w_sb[:], in_=w_d[:])
            if N_WARM:
                # Warm the PE HAM clock gate during the initial x DMA: matmuls
                # on a memset tile (no DMA dependency), results never read.
                warm_src = wp.tile([128, 504], dt, name="warm_src",
                                   tag="warm_src")
                nc.any.memset(warm_src[:], 0.0)
                warm_ps = pp.tile([128, 512], f32, name="warm", tag="ps0",
                                  bufs=PS_BUFS)
                for _wi in range(N_WARM):
                    nc.tensor.matmul(
                        warm_ps[:, 0:504], warm_src[0:64, 0:128],
                        warm_src[0:64, 0:504], start=True, stop=True)

            def copy0(ot_slice, ps_view):
                nc.vector.tensor_copy(ot_slice, ps_view)

            def copy1(ot_slice, ps_view):
                if COPY_MODE == "split":
                    nc.scalar.copy(ot_slice, ps_view)
                else:
                    nc.vector.tensor_copy(ot_slice, ps_view)

            for _rep in range(reps):
                x_sb = xp.tile([128, HW], dt, name="x_sb", tag="x")
                csz = HW // N_CHUNKS
                for ci in range(N_CHUNKS):
                    nc.sync.dma_start(
                        out=x_sb[:, ci * csz:(ci + 1) * csz],
                        in_=x_d[:, ci * csz:(ci + 1) * csz])
                xv = x_sb[:].rearrange("p (h w) -> p h w", h=H)

                def mm(ps, img, t, y, r):
                    ky, kx = divmod(t, 3)
                    p0 = img * 64
                    if RHS_MODE == "contig":
                        n = (r - 1) * W + WO
                        s = (y + ky) * W + kx
                        nc.tensor.matmul(
                            ps[:, 0:n],
                            w_sb[p0:p0 + 64, t * 128:(t + 1) * 128],
                            x_sb[p0:p0 + 64, s:s + n],
                            start=(t == 0), stop=(t == TAPS - 1),
                        )
                    else:
                        nc.tensor.matmul(
                            ps[:],
                            w_sb[p0:p0 + 64, t * 128:(t + 1) * 128],
                            xv[p0:p0 + 64, y + ky:y + ky + r, kx:kx + WO],
                            start=(t == 0), stop=(t == TAPS - 1),
                        )

                def ps_alloc(pool_tag, r, bufs):
                    if RHS_MODE == "contig":
                        t_ = pp.tile([128, 512], f32, name=pool_tag,
                                     tag=pool_tag, bufs=bufs)
                        return t_
                    return pp.tile([128, r * WO], f32, name=pool_tag,
                                   tag=pool_tag, bufs=bufs)

                def ps_read(ps, r):
                    if RHS_MODE == "contig":
                        return ps[:].rearrange(
                            "p (h w) -> p h w", w=W)[:, 0:r, 0:WO]
                    return ps[:]

                if ABLATE == "noout":
                    live = op.tile([128, 512], f32, name="live", tag="live")
                    k = 0
                    for y in range(0, HO, ROWS_PER_TILE):
                        r = min(ROWS_PER_TILE, HO - y)
                        ps0 = ps_alloc("ps0", r, PS_BUFS)
                        ps1 = ps_alloc("ps1", r, PS_BUFS)
                        for t in range(TAPS):
                            mm(ps0, 0, t, y, r)
                            mm(ps1, 1, t, y, r)
                        nc.vector.tensor_copy(live[:, k % 512:k % 512 + 1],
                                              ps0[:, 0:1])
                        k += 1
                        nc.vector.tensor_copy(live[:, k % 512:k % 512 + 1],
                                              ps1[:, 0:1])
                        k += 1
                    continue
                for y0 in range(0, HO, G_TILES * ROWS_PER_TILE):
                    rg = min(G_TILES * ROWS_PER_TILE, HO - y0)
                    ots = [op.tile([128, rg * WO], odt, name=f"ot{i}",
                                   tag=f"ot{i}", bufs=OUT_BUFS)
                           for i in range(IMGS_PER_CORE)]
                    ys = list(range(y0, y0 + rg, ROWS_PER_TILE))
                    if ORDER == "alt":
                        for y in ys:
                            r = min(ROWS_PER_TILE, HO - y)
                            ps0 = ps_alloc("ps0", r, PS_BUFS)
                            ps1 = ps_alloc("ps1", r, PS_BUFS)
                            for t in range(TAPS):
                                mm(ps0, 0, t, y, r)
                                mm(ps1, 1, t, y, r)
                            oc = (y - y0) * WO
                            copy0(ots[0][:, oc:oc + r * WO], ps_read(ps0, r))
                            copy1(ots[1][:, oc:oc + r * WO], ps_read(ps1, r))
                    else:  # tapgroup
                        for gi in range(0, len(ys), TAPGROUP):
                            sub = ys[gi:gi + TAPGROUP]
                            pairs = []
                            for y in sub:
                                r = min(ROWS_PER_TILE, HO - y)
                                ps0 = ps_alloc("ps0", r, PS_BUFS)
                                ps1 = ps_alloc("ps1", r, PS_BUFS)
                                pairs.append((y, r, ps0, ps1))
                            for t in range(TAPS):
                                for (y, r, ps0, ps1) in pairs:
                                    mm(ps0, 0, t, y, r)
                                    mm(ps1, 1, t, y, r)
                            for (y, r, ps0, ps1) in pairs:
                                oc = (y - y0) * WO
                                copy0(ots[0][:, oc:oc + r * WO],
                                      ps_read(ps0, r))
                                copy1(ots[1][:, oc:oc + r * WO],
                                      ps_read(ps1, r))
                    last_group = y0 + rg >= HO  # noqa: E501  (ablation skips via continue above)
                    for i in range(IMGS_PER_CORE):
                        if last_group and len(ys) > 1:
                            # split the final out DMA so its first half
                            # overlaps the last tiles' copies (shorter tail)
                            mid = ys[len(ys) // 2] - y0
                            nc.scalar.dma_start(
                                out=out_d[i, :, y0 * WO:(y0 + mid) * WO],
                                in_=ots[i][:, 0:mid * WO])
                            nc.scalar.dma_start(
                                out=out_d[i, :, (y0 + mid) * WO:(y0 + rg) * WO],
                                in_=ots[i][:, mid * WO:rg * WO])
                        else:
                            nc.scalar.dma_start(
                                out=out_d[i, :, y0 * WO:(y0 + rg) * WO],
                                in_=ots[i][:])
            if ABLATE == "noout":
                # satisfy the ExternalOutput with one token write
                tok = op.tile([128, 512], odt, name="tok", tag="tok")
                nc.any.memset(tok[:], 0.0)
                nc.sync.dma_start(out=out_d[0, :, 0:512], in_=tok[:])
    nc.compile()
    return nc


def _build_nop_program():
    """Minimal program with the same I/O contract, for dispatch-floor timing."""
    import concourse.bacc as bacc
    import concourse.mybir as mybir
    from concourse.tile import TileContext

    f32 = mybir.dt.float32
    nc = bacc.Bacc(None, target_bir_lowering=False)
    x_d = nc.dram_tensor("xn", [128, 128], f32, kind="ExternalInput")
    o_d = nc.dram_tensor("on", [128, 128], f32, kind="ExternalOutput")
    with TileContext(nc) as tc:
        with tc.tile_pool(name="p", bufs=1) as p:
            t = p.tile([128, 128], f32)
            nc.sync.dma_start(out=t[:], in_=x_d[:])
            nc.sync.dma_start(out=o_d[:], in_=t[:])
    nc.compile()
    return nc


def _make_runner(nc):
    """Build a reusable jitted SPMD callable for `nc` over 8 cores.

    Returns (run, meta): run(list_of_global_np_inputs) -> list of global
    np outputs with shape (N_CORES*dim0, ...). Inputs are device_put once
    per call; no donation (outputs fully written by the kernel).
    """
    import jax
    import concourse.mybir as mybir
    from concourse import bass2jax
    from jax.experimental.shard_map import shard_map
    from jax.sharding import Mesh, NamedSharding, PartitionSpec

    bass2jax.install_neuronx_cc_hook()

    partition_name = (
        nc.partition_id_tensor.name if nc.partition_id_tensor is not None else None
    )
    in_names, out_names, out_avals, zero_outs = [], [], [], []
    for alloc in nc.m.functions[0].allocations:
        if not isinstance(alloc, mybir.MemoryLocationSet):
            continue
        name = alloc.memorylocations[0].name
        if alloc.kind == "ExternalInput":
            if name != partition_name:
                in_names.append(name)
        elif alloc.kind == "ExternalOutput":
            out_names.append(name)
            shape = tuple(alloc.tensor_shape)
            dtype = mybir.dt.np(alloc.dtype)
            out_avals.append(jax.core.ShapedArray(shape, dtype))
            zero_outs.append(np.zeros(shape, dtype))
    n_params = len(in_names)
    all_in_names = list(in_names) + list(out_names)
    if partition_name is not None:
        all_in_names.append(partition_name)

    def _body(*args):
        operands = list(args)
        if partition_name is not None:
            operands.append(bass2jax.partition_id_tensor())
        outs = bass2jax._bass_exec_p.bind(
            *operands,
            out_avals=tuple(out_avals),
            in_names=tuple(all_in_names),
            out_names=tuple(out_names),
            lowering_input_output_aliases=(),
            sim_require_finite=True,
            sim_require_nnan=True,
            nc=nc,
        )
        return tuple(outs)

    devices = jax.devices()[:N_CORES]
    mesh = Mesh(np.asarray(devices), ("core",))
    spec = PartitionSpec("core")
    n_args = n_params + len(out_names)
    sharded = jax.jit(
        shard_map(
            _body,
            mesh=mesh,
            in_specs=(spec,) * n_args,
            out_specs=(spec,) * len(out_names),
            check_rep=False,
        ),
        keep_unused=True,
    )
    sharding = NamedSharding(mesh, spec)
    zeros_dev = [
        jax.device_put(np.zeros((N_CORES * z.shape[0], *z.shape[1:]), z.dtype),
                       sharding)
        for z in zero_outs
    ]

    def run(global_inputs, device_inputs=None):
        if device_inputs is None:
            device_inputs = [jax.device_put(g, sharding) for g in global_inputs]
        outs = sharded(*device_inputs, *zeros_dev)
        jax.block_until_ready(outs)
        return outs

    meta = {
        "sharding": sharding,
        "out_avals": out_avals,
        "out_names": out_names,
        "jax": jax,
        "sharded": sharded,
        "zeros_dev": zeros_dev,
    }
    return run, meta


def get_runner(reps=1):
    key = ("runner", reps)
    if key not in _CACHE:
        nc = _build_program(reps)
        _CACHE[key] = _make_runner(nc)
    return _CACHE[key]


def get_nop_runner():
    if "nop_runner" not in _CACHE:
        nc = _build_nop_program()
        _CACHE["nop_runner"] = _make_runner(nc)
    return _CACHE["nop_runner"]


def _np_mm_dtype():
    return {"float16": np.float16, "bfloat16": None}.get(MM_DTYPE, np.float32)


def prep_inputs(x, weights):
    """Host-side shard prep: returns global (concat over cores) inputs."""
    npdt = _np_mm_dtype()
    if npdt is None:
        import ml_dtypes
        npdt = ml_dtypes.bfloat16
    x = np.asarray(x, dtype=np.float32).reshape(B, C_IN, HW).astype(npdt)
    w = np.asarray(weights, dtype=np.float32).astype(npdt)  # [128, 64, 9]

    if SCHEME == "dup128":
        # per image: upper 64 partitions = channels, lower 64 = shifted +1
        x2 = np.zeros((B, 2 * C_IN, HW), npdt)
        x2[:, :C_IN, :] = x
        x2[:, C_IN:, :HW - 1] = x[:, :, 1:]
        NS = len(SLOTS128)
        wmm = np.zeros((128, NS * 128), npdt)
        for s, (_off, tu, tl) in enumerate(SLOTS128):
            wmm[0:64, s * 128:(s + 1) * 128] = w[:, :, tu].T
            if tl is not None:
                wmm[64:128, s * 128:(s + 1) * 128] = w[:, :, tl].T
        wmm_global = np.tile(wmm, (N_CORES, 1))
        # x_d is [2, 128, HW] per core -> global keeps trailing dims:
        # [16, 128, HW], dim0 sharded 8-ways into pairs of images
        return [x2, wmm_global]

    # core c holds images 2c (partitions 0-63) and 2c+1 (64-127)
    x2_global = np.ascontiguousarray(
        x.reshape(N_CORES, IMGS_PER_CORE * C_IN, HW))
    wmm = np.zeros((128, TAPS * 128), npdt)
    for t in range(TAPS):
        wT = w[:, :, t].T  # [64, 128]
        wmm[0:64, t * 128:(t + 1) * 128] = wT
        wmm[64:128, t * 128:(t + 1) * 128] = wT
    wmm_global = np.tile(wmm, (N_CORES, 1))  # [8*128, 1152]
    return [x2_global.reshape(N_CORES * 2 * C_IN, HW), wmm_global]


def kernel(x, weights):
    run, _meta = get_runner()
    outs = run(prep_inputs(x, weights))
    out_g = np.asarray(outs[0])  # [16, 128, HO*WO] in OUT_DTYPE
    return out_g.reshape(B, C_OUT, HO, WO).astype(np.float32)


# revision 24
# speedup vs baseline: 2.7432x; 2.1879x over previous
"""Trainium2 Bass kernel for 3x3 VALID conv (nn_BreakupConv).

x [16,64,128,128] f32, weights [128,64,9] f32 -> out [16,128,126,126] f32.

Scheme (v2c, "image-split row tiling, contiguous moving operand"):
- Data-parallel over 8 cores: 2 images per core.
- Per core, one SBUF x tile [128, 16384] fp16: partitions 0-63 hold image
  A's 64 channels, partitions 64-127 image B's. Every conv tap (ky,kx) is
  a K=64 matmul reading this UNSHIFTED buffer: image A on PE row group
  (0,0), image B on (64,0). Consecutive A/B matmuls use disjoint PE row
  halves and accumulate into different PSUM banks, so the hardware runs
  them concurrently (row tiling): a tap pair streams in one span. 9 taps
  covers TWO output tiles = the 4.5-span/128-row minimum for the 576-long
  contraction.
- The moving operand is a CONTIGUOUS 1D span of (r-1)*128+126 elements
  starting at flat offset (y+ky)*128+kx: it covers the r x 126 output
  window plus 6 junk columns in the inter-row gaps, positionally aligned
  across taps so PSUM accumulation stays correct. Contiguity matters: a
  strided 3D rhs ([64, r, 126] stride 128) streams ~2.3x slower on the PE
  (measured 88.7us vs 38us per rep in isolation); contiguous fp16 streams
  ~2 elements/cycle. The junk columns are dropped by the PSUM->SBUF copy
  (strided DVE read).
- PSUM banks drained to SBUF as fp16, DMA'd out in ~2 MiB groups (the
  final group's DMA is split in half so the tail overlaps the last
  copies). 16 warmup matmuls on a memset tile bridge the PE HAM
  clock-gate window during the initial x DMA (single-shot latency; they
  have no DMA dependency and are never read).
- I/O per core: 4 MiB in + 8.1 MiB out (fp16) vs 8.4 + 16.3 MiB for v1.
- Measured (R=199 paired-reps): ~29 us/rep vs ~190-210 us for v1 and for
  the strided variant; at the fp16 PE streaming peak (~30 us model,
  ~157 TFLOP/s effective) with DMA fully overlapped. fp8 would breach
  the 2e-2 error gate (measured 3.8e-2 host-side), so fp16 is final.
"""

import os
import numpy as np

os.environ.setdefault("BASS_NEVER_TRACE", "1")

B, C_IN, H, W = 16, 64, 128, 128
C_OUT, HO, WO = 128, 126, 126
N_CORES = 8
IMGS_PER_CORE = B // N_CORES  # 2
HW = H * W                    # 16384
ROWS_PER_TILE = 4             # output rows per PSUM tile (4*126 = 504 <= 512)
TAPS = 9

MM_DTYPE = os.environ.get("CONV_MM_DTYPE", "float16")
OUT_DTYPE = os.environ.get("CONV_OUT_DT", "float16")
# "alt": per tile pair, taps interleaved A,B,A,B -> relies on the PE
#   background weight buffer to hide per-matmul LDWEIGHTS.
# "tapgroup": tap-outer over TAPGROUP tile pairs -> same-weight matmuls
#   run back-to-back so weight (re)loads amortize over the group.
ORDER = os.environ.get("CONV_ORDER", "alt")
# "strided": matmul rhs is the 3D [64, r, 126] stride-128 window view.
# "contig": rhs is one contiguous span of (r-1)*128+126 elements covering
#   the window plus 6 junk columns (positionally consistent across taps);
#   the PSUM->SBUF copy drops the junk via a strided view. +1.2% PE cols,
#   but a pure 1D moving access pattern.
RHS_MODE = os.environ.get("CONV_RHS", "contig")
TAPGROUP = int(os.environ.get("CONV_TAPGROUP", "3"))
N_CHUNKS = int(os.environ.get("CONV_CHUNKS", "8"))
G_TILES = int(os.environ.get("CONV_G", "16"))   # row tiles per out DMA
N_WARM = int(os.environ.get("CONV_WARMUP", "16"))  # PE warmup matmuls
COPY_MODE = os.environ.get("CONV_COPY", "dve")  # "dve" | "split"
PS_BUFS = int(os.environ.get("CONV_PS_BUFS", "4"))   # PSUM bufs per image
# 3 bufs per image: the copy into out-slot g+2 then never waits on group
# g's 2 MiB DMA — matters when HBM contention stretches the transfer past
# a group's ~14.5 us compute span. SBUF: 3*2*16.1KB + x 64KB fits in 208.
OUT_BUFS = int(os.environ.get("CONV_OUT_BUFS", "3"))  # out bufs per image
# Timing-only ablation (breaks correctness): "noout" drops the output
# path (copies become 1-column live-sinks, no output DMAs).
ABLATE = os.environ.get("CONV_ABLATE", "")
# "pair": K=64 image-split row tiling (default). "dup128": v0-style
# shifted-dup layout (upper 64 partitions = image, lower = image shifted
# +1 col) with K=128 matmuls, 6 slots/tile, contiguous rhs. Measured
# (R=199): dup128 = 80.8 us/rep vs pair = 28.3 — a K=128 matmul streams
# ~1 elem/cycle (~210 ns/MM) while K=64 row-tiled pairs reach ~49 ns
# effective per MM (2-elem/cycle fp16 streaming x 2 concurrent row
# groups). Keep "pair".
SCHEME = os.environ.get("CONV_SCHEME", "pair")
# slot -> (read offset (ky,kx), upper tap, lower tap); lower half holds
# x shifted +1 col, so its effective tap is (ky, kx+1).
SLOTS128 = [
    ((0, 0), 0, 1), ((1, 0), 3, 4), ((2, 0), 6, 7),
    ((0, 2), 2, None), ((1, 2), 5, None), ((2, 2), 8, None),
]

_CACHE = {}


def _build_dup128(reps=1):
    """v0-style shifted-dup layout with K=128 matmuls and contiguous rhs."""
    import concourse.bacc as bacc
    import concourse.mybir as mybir
    from concourse.tile import TileContext

    dt = getattr(mybir.dt, MM_DTYPE)
    odt = getattr(mybir.dt, OUT_DTYPE)
    f32 = mybir.dt.float32
    NS = len(SLOTS128)

    nc = bacc.Bacc(None, target_bir_lowering=False)
    x_d = nc.dram_tensor("x2", [IMGS_PER_CORE, 128, HW], dt,
                         kind="ExternalInput")
    w_d = nc.dram_tensor("wmm", [128, NS * 128], dt, kind="ExternalInput")
    out_d = nc.dram_tensor("out2", [IMGS_PER_CORE, C_OUT, HO * WO], odt,
                           kind="ExternalOutput")

    with TileContext(nc) as tc:
        with (
            tc.tile_pool(name="xp", bufs=2) as xp,
            tc.tile_pool(name="wp", bufs=1) as wp,
            tc.tile_pool(name="pp", bufs=8, space="PSUM") as pp,
            tc.tile_pool(name="op", bufs=2 * OUT_BUFS) as op,
        ):
            w_sb = wp.tile([128, NS * 128], dt)
            nc.sync.dma_start(out=w_sb[:], in_=w_d[:])
            if N_WARM:
                warm_src = wp.tile([128, 504], dt, name="warm_src",
                                   tag="warm_src")
                nc.any.memset(warm_src[:], 0.0)
                warm_ps = pp.tile([128, 512], f32, name="warm", tag="ps",
                                  bufs=8)
                for _wi in range(N_WARM):
                    nc.tensor.matmul(
                        warm_ps[:, 0:504], warm_src[0:64, 0:128],
                        warm_src[0:64, 0:504], start=True, stop=True)
            for _rep in range(reps):
                for img in range(IMGS_PER_CORE):
                    x_sb = xp.tile([128, HW], dt, name="x_sb",
                                   tag=f"x{img}", bufs=1)
                    csz = HW // N_CHUNKS
                    for ci in range(N_CHUNKS):
                        nc.sync.dma_start(
                            out=x_sb[:, ci * csz:(ci + 1) * csz],
                            in_=x_d[img, :, ci * csz:(ci + 1) * csz])
                    for y0 in range(0, HO, G_TILES * ROWS_PER_TILE):
                        rg = min(G_TILES * ROWS_PER_TILE, HO - y0)
                        ot = op.tile([128, rg * WO], odt, name="ot",
                                     tag="ot", bufs=2 * OUT_BUFS)
                        ys = list(range(y0, y0 + rg, ROWS_PER_TILE))
                        for y in ys:
                            r = min(ROWS_PER_TILE, HO - y)
                            ps = pp.tile([128, 512], f32, name="ps",
                                         tag="ps", bufs=8)
                            n = (r - 1) * W + WO
                            for s, ((ky, kx), _tu, _tl) in enumerate(SLOTS128):
                                st = (y + ky) * W + kx
                                nc.tensor.matmul(
                                    ps[:, 0:n],
                                    w_sb[:, s * 128:(s + 1) * 128],
                                    x_sb[:, st:st + n],
                                    start=(s == 0), stop=(s == NS - 1))
                            oc = (y - y0) * WO
                            psv = ps[:].rearrange(
                                "p (h w) -> p h w", w=W)[:, 0:r, 0:WO]
                            nc.vector.tensor_copy(ot[:, oc:oc + r * WO], psv)
                        last_group = y0 + rg >= HO
                        if last_group and len(ys) > 1:
                            mid = ys[len(ys) // 2] - y0
                            nc.sync.dma_start(
                                out=out_d[img, :, y0 * WO:(y0 + mid) * WO],
                                in_=ot[:, 0:mid * WO])
                            nc.sync.dma_start(
                                out=out_d[img, :,
                                          (y0 + mid) * WO:(y0 + rg) * WO],
                                in_=ot[:, mid * WO:rg * WO])
                        else:
                            nc.sync.dma_start(
                                out=out_d[img, :, y0 * WO:(y0 + rg) * WO],
                                in_=ot[:])
    nc.compile()
    return nc


def _build_program(reps=1):
    import concourse.bacc as bacc
    import concourse.mybir as mybir
    from concourse.tile import TileContext

    if SCHEME == "dup128":
        return _build_dup128(reps)

    dt = getattr(mybir.dt, MM_DTYPE)
    odt = getattr(mybir.dt, OUT_DTYPE)
    f32 = mybir.dt.float32

    nc = bacc.Bacc(None, target_bir_lowering=False)
    x_d = nc.dram_tensor("x2", [128, HW], dt, kind="ExternalInput")
    w_d = nc.dram_tensor("wmm", [128, TAPS * 128], dt, kind="ExternalInput")
    out_d = nc.dram_tensor("out2", [IMGS_PER_CORE, C_OUT, HO * WO], odt,
                           kind="ExternalOutput")

    with TileContext(nc) as tc:
        with (
            tc.tile_pool(name="xp", bufs=2) as xp,
            tc.tile_pool(name="wp", bufs=1) as wp,
            tc.tile_pool(name="pp", bufs=2 * PS_BUFS, space="PSUM") as pp,
            tc.tile_pool(name="op", bufs=2 * OUT_BUFS) as op,
        ):
            w_sb = wp.tile([128, TAPS * 128], dt)
            # ACT HWDGE ring (qActDynamicHW): keeps the weights load and all
            # output DMAs off the SP ring's FIFO so input chunks are never
            # head-of-line blocked behind 2 MiB output transfers.
            nc.scalar.dma_start(out=w_sb[:], in_=w_d[:])
            if N_WARM:
                # Warm the PE HAM clock gate during the initial x DMA: matmuls
                # on a memset tile (no DMA dependency), results never read.
                warm_src = wp.tile([128, 504], dt, name="warm_src",
                                   tag="warm_src")
                nc.any.memset(warm_src[:], 0.0)
                warm_ps = pp.tile([128, 512], f32, name="warm", tag="ps0",
                                  bufs=PS_BUFS)
                for _wi in range(N_WARM):
                    nc.tensor.matmul(
                        warm_ps[:, 0:504], warm_src[0:64, 0:128],
                        warm_src[0:64, 0:504], start=True, stop=True)

            def copy0(ot_slice, ps_view):
                nc.vector.tensor_copy(ot_slice, ps_view)

            def copy1(ot_slice, ps_view):
                if COPY_MODE == "split":
                    nc.scalar.copy(ot_slice, ps_view)
                else:
                    nc.vector.tensor_copy(ot_slice, ps_view)

            for _rep in range(reps):
                x_sb = xp.tile([128, HW], dt, name="x_sb", tag="x")
                csz = HW // N_CHUNKS
                for ci in range(N_CHUNKS):
                    nc.sync.dma_start(
                        out=x_sb[:, ci * csz:(ci + 1) * csz],
                        in_=x_d[:, ci * csz:(ci + 1) * csz])
                xv = x_sb[:].rearrange("p (h w) -> p h w", h=H)

                def mm(ps, img, t, y, r):
                    ky, kx = divmod(t, 3)
                    p0 = img * 64
                    if RHS_MODE == "contig":
                        n = (r - 1) * W + WO
                        s = (y + ky) * W + kx
                        nc.tensor.matmul(
                            ps[:, 0:n],
                            w_sb[p0:p0 + 64, t * 128:(t + 1) * 128],
                            x_sb[p0:p0 + 64, s:s + n],
                            start=(t == 0), stop=(t == TAPS - 1),
                        )
                    else:
                        nc.tensor.matmul(
                            ps[:],
                            w_sb[p0:p0 + 64, t * 128:(t + 1) * 128],
                            xv[p0:p0 + 64, y + ky:y + ky + r, kx:kx + WO],
                            start=(t == 0), stop=(t == TAPS - 1),
                        )

                def ps_alloc(pool_tag, r, bufs):
                    if RHS_MODE == "contig":
                        t_ = pp.tile([128, 512], f32, name=pool_tag,
                                     tag=pool_tag, bufs=bufs)
                        return t_
                    return pp.tile([128, r * WO], f32, name=pool_tag,
                                   tag=pool_tag, bufs=bufs)

                def ps_read(ps, r):
                    if RHS_MODE == "contig":
                        return ps[:].rearrange(
                            "p (h w) -> p h w", w=W)[:, 0:r, 0:WO]
                    return ps[:]

                if ABLATE == "noout":
                    live = op.tile([128, 512], f32, name="live", tag="live")
                    k = 0
                    for y in range(0, HO, ROWS_PER_TILE):
                        r = min(ROWS_PER_TILE, HO - y)
                        ps0 = ps_alloc("ps0", r, PS_BUFS)
                        ps1 = ps_alloc("ps1", r, PS_BUFS)
                        for t in range(TAPS):
                            mm(ps0, 0, t, y, r)
                            mm(ps1, 1, t, y, r)
                        nc.vector.tensor_copy(live[:, k % 512:k % 512 + 1],
                                              ps0[:, 0:1])
                        k += 1
                        nc.vector.tensor_copy(live[:, k % 512:k % 512 + 1],
                                              ps1[:, 0:1])
                        k += 1
                    continue
                for y0 in range(0, HO, G_TILES * ROWS_PER_TILE):
                    rg = min(G_TILES * ROWS_PER_TILE, HO - y0)
                    ots = [op.tile([128, rg * WO], odt, name=f"ot{i}",
                                   tag=f"ot{i}", bufs=OUT_BUFS)
                           for i in range(IMGS_PER_CORE)]
                    ys = list(range(y0, y0 + rg, ROWS_PER_TILE))
                    if ORDER == "alt":
                        for y in ys:
                            r = min(ROWS_PER_TILE, HO - y)
                            ps0 = ps_alloc("ps0", r, PS_BUFS)
                            ps1 = ps_alloc("ps1", r, PS_BUFS)
                            for t in range(TAPS):
                                mm(ps0, 0, t, y, r)
                                mm(ps1, 1, t, y, r)
                            oc = (y - y0) * WO
                            copy0(ots[0][:, oc:oc + r * WO], ps_read(ps0, r))
                            copy1(ots[1][:, oc:oc + r * WO], ps_read(ps1, r))
                    else:  # tapgroup
                        for gi in range(0, len(ys), TAPGROUP):
                            sub = ys[gi:gi + TAPGROUP]
                            pairs = []
                            for y in sub:
                                r = min(ROWS_PER_TILE, HO - y)
                                ps0 = ps_alloc("ps0", r, PS_BUFS)
                                ps1 = ps_alloc("ps1", r, PS_BUFS)
                                pairs.append((y, r, ps0, ps1))
                            for t in range(TAPS):
                                for (y, r, ps0, ps1) in pairs:
                                    mm(ps0, 0, t, y, r)
                                    mm(ps1, 1, t, y, r)
                            for (y, r, ps0, ps1) in pairs:
                                oc = (y - y0) * WO
                                copy0(ots[0][:, oc:oc + r * WO],
                                      ps_read(ps0, r))
                                copy1(ots[1][:, oc:oc + r * WO],
                                      ps_read(ps1, r))
                    last_group = y0 + rg >= HO  # noqa: E501  (ablation skips via continue above)
                    for i in range(IMGS_PER_CORE):
                        if last_group and len(ys) > 1:
                            # split the final out DMA so its first half
                            # overlaps the last tiles' copies (shorter tail)
                            mid = ys[len(ys) // 2] - y0
                            nc.scalar.dma_start(
                                out=out_d[i, :, y0 * WO:(y0 + mid) * WO],
                                in_=ots[i][:, 0:mid * WO])
                            nc.scalar.dma_start(
                                out=out_d[i, :, (y0 + mid) * WO:(y0 + rg) * WO],
                                in_=ots[i][:, mid * WO:rg * WO])
                        else:
                            nc.scalar.dma_start(
                                out=out_d[i, :, y0 * WO:(y0 + rg) * WO],
                                in_=ots[i][:])
            if ABLATE == "noout":
                # satisfy the ExternalOutput with one token write
                tok = op.tile([128, 512], odt, name="tok", tag="tok")
                nc.any.memset(tok[:], 0.0)
                nc.sync.dma_start(out=out_d[0, :, 0:512], in_=tok[:])
    nc.compile()
    return nc


def _build_nop_program():
    """Minimal program with the same I/O contract, for dispatch-floor timing."""
    import concourse.bacc as bacc
    import concourse.mybir as mybir
    from concourse.tile import TileContext

    f32 = mybir.dt.float32
    nc = bacc.Bacc(None, target_bir_lowering=False)
    x_d = nc.dram_tensor("xn", [128, 128], f32, kind="ExternalInput")
    o_d = nc.dram_tensor("on", [128, 128], f32, kind="ExternalOutput")
    with TileContext(nc) as tc:
        with tc.tile_pool(name="p", bufs=1) as p:
            t = p.tile([128, 128], f32)
            nc.sync.dma_start(out=t[:], in_=x_d[:])
            nc.sync.dma_start(out=o_d[:], in_=t[:])
    nc.compile()
    return nc


def _make_runner(nc):
    """Build a reusable jitted SPMD callable for `nc` over 8 cores.

    Returns (run, meta): run(list_of_global_np_inputs) -> list of global
    np outputs with shape (N_CORES*dim0, ...). Inputs are device_put once
    per call; no donation (outputs fully written by the kernel).
    """
    import jax
    import concourse.mybir as mybir
    from concourse import bass2jax
    from jax.experimental.shard_map import shard_map
    from jax.sharding import Mesh, NamedSharding, PartitionSpec

    bass2jax.install_neuronx_cc_hook()

    partition_name = (
        nc.partition_id_tensor.name if nc.partition_id_tensor is not None else None
    )
    in_names, out_names, out_avals, zero_outs = [], [], [], []
    for alloc in nc.m.functions[0].allocations:
        if not isinstance(alloc, mybir.MemoryLocationSet):
            continue
        name = alloc.memorylocations[0].name
        if alloc.kind == "ExternalInput":
            if name != partition_name:
                in_names.append(name)
        elif alloc.kind == "ExternalOutput":
            out_names.append(name)
            shape = tuple(alloc.tensor_shape)
            dtype = mybir.dt.np(alloc.dtype)
            out_avals.append(jax.core.ShapedArray(shape, dtype))
            zero_outs.append(np.zeros(shape, dtype))
    n_params = len(in_names)
    all_in_names = list(in_names) + list(out_names)
    if partition_name is not None:
        all_in_names.append(partition_name)

    def _body(*args):
        operands = list(args)
        if partition_name is not None:
            operands.append(bass2jax.partition_id_tensor())
        outs = bass2jax._bass_exec_p.bind(
            *operands,
            out_avals=tuple(out_avals),
            in_names=tuple(all_in_names),
            out_names=tuple(out_names),
            lowering_input_output_aliases=(),
            sim_require_finite=True,
            sim_require_nnan=True,
            nc=nc,
        )
        return tuple(outs)

    devices = jax.devices()[:N_CORES]
    mesh = Mesh(np.asarray(devices), ("core",))
    spec = PartitionSpec("core")
    n_args = n_params + len(out_names)
    sharded = jax.jit(
        shard_map(
            _body,
            mesh=mesh,
            in_specs=(spec,) * n_args,
            out_specs=(spec,) * len(out_names),
            check_rep=False,
        ),
        keep_unused=True,
    )
    sharding = NamedSharding(mesh, spec)
    zeros_dev = [
        jax.device_put(np.zeros((N_CORES * z.shape[0], *z.shape[1:]), z.dtype),
                       sharding)
        for z in zero_outs
    ]

    def run(global_inputs, device_inputs=None):
        if device_inputs is None:
            device_inputs = [jax.device_put(g, sharding) for g in global_inputs]
        outs = sharded(*device_inputs, *zeros_dev)
        jax.block_until_ready(outs)
        return outs

    meta = {
        "sharding": sharding,
        "out_avals": out_avals,
        "out_names": out_names,
        "jax": jax,
        "sharded": sharded,
        "zeros_dev": zeros_dev,
    }
    return run, meta


def get_runner(reps=1):
    key = ("runner", reps)
    if key not in _CACHE:
        nc = _build_program(reps)
        _CACHE[key] = _make_runner(nc)
    return _CACHE[key]


def get_nop_runner():
    if "nop_runner" not in _CACHE:
        nc = _build_nop_program()
        _CACHE["nop_runner"] = _make_runner(nc)
    return _CACHE["nop_runner"]


def _np_mm_dtype():
    return {"float16": np.float16, "bfloat16": None}.get(MM_DTYPE, np.float32)


def prep_inputs(x, weights):
    """Host-side shard prep: returns global (concat over cores) inputs."""
    npdt = _np_mm_dtype()
    if npdt is None:
        import ml_dtypes
        npdt = ml_dtypes.bfloat16
    x = np.asarray(x, dtype=np.float32).reshape(B, C_IN, HW).astype(npdt)
    w = np.asarray(weights, dtype=np.float32).astype(npdt)  # [128, 64, 9]

    if SCHEME == "dup128":
        # per image: upper 64 partitions = channels, lower 64 = shifted +1
        x2 = np.zeros((B, 2 * C_IN, HW), npdt)
        x2[:, :C_IN, :] = x
        x2[:, C_IN:, :HW - 1] = x[:, :, 1:]
        NS = len(SLOTS128)
        wmm = np.zeros((128, NS * 128), npdt)
        for s, (_off, tu, tl) in enumerate(SLOTS128):
            wmm[0:64, s * 128:(s + 1) * 128] = w[:, :, tu].T
            if tl is not None:
                wmm[64:128, s * 128:(s + 1) * 128] = w[:, :, tl].T
        wmm_global = np.tile(wmm, (N_CORES, 1))
        # x_d is [2, 128, HW] per core -> global keeps trailing dims:
        # [16, 128, HW], dim0 sharded 8-ways into pairs of images
        return [x2, wmm_global]

    # core c holds images 2c (partitions 0-63) and 2c+1 (64-127)
    x2_global = np.ascontiguousarray(
        x.reshape(N_CORES, IMGS_PER_CORE * C_IN, HW))
    wmm = np.zeros((128, TAPS * 128), npdt)
    for t in range(TAPS):
        wT = w[:, :, t].T  # [64, 128]
        wmm[0:64, t * 128:(t + 1) * 128] = wT
        wmm[64:128, t * 128:(t + 1) * 128] = wT
    wmm_global = np.tile(wmm, (N_CORES, 1))  # [8*128, 1152]
    return [x2_global.reshape(N_CORES * 2 * C_IN, HW), wmm_global]


def kernel(x, weights):
    run, _meta = get_runner()
    outs = run(prep_inputs(x, weights))
    out_g = np.asarray(outs[0])  # [16, 128, HO*WO] in OUT_DTYPE
    return out_g.reshape(B, C_OUT, HO, WO).astype(np.float32)
